# revision 11
# baseline (speedup 1.0000x reference)
"""BitNet FFN Trainium2 kernel (8-core SPMD, data-parallel over tokens).

Math (forward values of the STE reference):
  wq(w)  = clip(round(w/s), -1, 1) * s,  s = mean(|w|) + EPS        (ternary)
  xq(x)  = round(x/sx) * sx,  sx = max(absmax_row(x), EPS)/127      (int8 range)
  gate = sigmoid(xq @ wq_g.T); up = xq @ wq_u.T; h = gate*up
  out  = hq(h) @ wq_d.T

Strategy: every matmul runs in bf16 with fp32 PSUM accumulation on exact
integers (|int| <= 127 activations, ternary weights, partial sums < 2^24),
so the integer matmuls are exact; all scales are folded in fp32 outside the
matmuls. Tokens are sharded 8 ways (1024/core); each core streams the full
weights once. The only collective is a 16-byte AllReduce for the three
global weight-scale sums.
"""

import sys

sys.path.insert(0, "/opt/trn_rl_repo")

import numpy as np

import concourse.bass as bass
import concourse.tile as tile
from concourse import bacc, mybir

F32 = mybir.dt.float32
BF16 = mybir.dt.bfloat16
ADD = mybir.AluOpType.add
SUB = mybir.AluOpType.subtract
MULT = mybir.AluOpType.mult
MAX = mybir.AluOpType.max
AXX = mybir.AxisListType.X
AFT = mybir.ActivationFunctionType

EPS = 1e-5
CR = 12582912.0  # 1.5*2^23: fp32 RNE round-to-integer magic constant
ALPHA = 1.0986122886681098  # atanh(0.5)/0.5 : tanh(ALPHA*0.5) == 0.5
P = 128


def build_program(T, DM, FF, ncores, ff_sh, dm_sh):
    """Build the per-core SPMD program.

    T: tokens per core; DM: d_model; FF: d_ff; ff_sh/dm_sh: rows of the
    per-core weight-scale shards (w_gate/w_up shard rows, w_down shard rows).
    """
    assert T % P == 0 and DM % P == 0 and FF % 1024 == 0
    MT = T // P              # token tiles
    KD = DM // P             # d_model k-blocks
    NG = FF // 1024          # phase-1 ff groups (8 strips each)
    K3 = FF // P             # phase-3 ff k-blocks
    KG = 16 if K3 % 16 == 0 else K3  # phase-3 k-group size
    NKG = K3 // KG
    MD = DM // P             # output dm blocks
    MGP1 = min(4, MT)        # phase-1 token-tile group
    MGP3 = min(4, MD)        # phase-3 dm-block group
    TN = min(512, T)         # moving free dim (tokens) per matmul
    NT3 = T // TN            # phase-3 token chunks
    WPC = min(2048, DM)      # scale-pass piece width for g/u
    WPC3 = min(2048, FF)     # scale-pass piece width for wd

    nc = bacc.Bacc(
        "TRN2",
        target_bir_lowering=False,
        debug=False,
        enable_asserts=False,
        num_devices=ncores,
    )

    x_d = nc.dram_tensor("x", [T, DM], F32, kind="ExternalInput")
    wg_d = nc.dram_tensor("wg", [FF, DM], F32, kind="ExternalInput")
    wu_d = nc.dram_tensor("wu", [FF, DM], F32, kind="ExternalInput")
    wd_d = nc.dram_tensor("wd", [DM, FF], F32, kind="ExternalInput")
    wgs_d = nc.dram_tensor("wg_sh", [ff_sh, DM], F32, kind="ExternalInput")
    wus_d = nc.dram_tensor("wu_sh", [ff_sh, DM], F32, kind="ExternalInput")
    wds_d = nc.dram_tensor("wd_sh", [dm_sh, FF], F32, kind="ExternalInput")
    out_d = nc.dram_tensor("out_t", [DM, T], F32, kind="ExternalOutput")

    NW = float(FF * DM)  # elements per weight matrix (all three equal)

    with tile.TileContext(nc, num_cores=ncores) as tc:
        import contextlib

        with contextlib.ExitStack() as outer:
            dram = outer.enter_context(tc.tile_pool(name="dram", bufs=1, space="DRAM"))
            psum = outer.enter_context(tc.tile_pool(name="psum", bufs=8, space="PSUM"))
            tiny = outer.enter_context(tc.tile_pool(name="tiny", bufs=1))

            hp_d = dram.tile([T, FF], F32)       # h' = sigmoid(G)*U_int
            hq_d = dram.tile([T, FF], BF16)      # quantized h ints
            wdt_d = dram.tile([DM, FF], BF16)    # ternary w_down (natural layout)
            shs_d = dram.tile([1, T], F32)       # per-token output scale row
            cc_in = dram.tile([1, 4], F32)
            cc_out = dram.tile([1, 4], F32)

            # persistent small tiles
            ones_col = tiny.tile([P, 1], F32)
            nc.vector.memset(ones_col, 1.0)
            ones_row = tiny.tile([1, P], F32)
            nc.vector.memset(ones_row, 1.0)
            sb_scales = tiny.tile([P, 8], F32)   # bcast: bg,bu,bd,swg,swu,swd
            sx_all = tiny.tile([P, MT], F32)     # per-token x scale (col=token tile)
            rx_all = tiny.tile([P, MT], F32)
            sxg_all = tiny.tile([P, MT], F32)    # sx*swg (sigmoid input scale)
            sxu_all = tiny.tile([P, MT], F32)    # sx*swu
            rph_all = tiny.tile([P, MT], F32)    # s_xu/s_h (h' quant scale)
            shd_all = tiny.tile([P, MT], F32)    # s_h*s_wd (output scale)
            accs = tiny.tile([P, MT, 2 * NG], F32)  # h' absmax partials

            # ---------------- S0: global weight scales ----------------
            with tc.tile_pool(name="s0", bufs=3) as s0p, tc.tile_pool(
                name="s0t", bufs=4
            ) as s0t:
                acc3 = tiny.tile([P, 4], F32)
                nc.vector.memset(acc3, 0.0)
                shard_specs = [
                    (wgs_d, 0, ff_sh, DM, WPC),
                    (wus_d, 1, ff_sh, DM, WPC),
                    (wds_d, 2, dm_sh, FF, WPC3),
                ]
                for src, col, rows, cols, pw in shard_specs:
                    for r0 in range(0, rows, P):
                        pr = min(P, rows - r0)
                        for c0 in range(0, cols, pw):
                            t_in = s0p.tile([P, pw], F32, name="s0raw")
                            nc.sync.dma_start(
                                t_in[:pr], src[r0 : r0 + pr, c0 : c0 + pw]
                            )
                            t_abs = s0p.tile([P, pw], F32, name="s0abs")
                            t_sum = s0t.tile([P, 1], F32, name="s0sum")
                            nc.scalar.activation(
                                out=t_abs[:pr],
                                in_=t_in[:pr],
                                func=AFT.Abs,
                                accum_out=t_sum[:pr],
                            )
                            nc.vector.tensor_tensor(
                                out=acc3[:pr, col : col + 1],
                                in0=acc3[:pr, col : col + 1],
                                in1=t_sum[:pr],
                                op=ADD,
                            )
                ps_s = psum.tile([P, 512], F32, name="ps_main")
                nc.tensor.matmul(
                    ps_s[:4, :1], acc3[:, :4], ones_col, start=True, stop=True
                )
                sb_s = s0t.tile([4, 1], F32, name="sb_s")
                nc.vector.tensor_copy(sb_s, ps_s[:4, :1])
                nc.sync.dma_start(cc_in[0, :4], sb_s[:, 0])
                nc.gpsimd.collective_compute(
                    "AllReduce",
                    ADD,
                    replica_groups=[list(range(ncores))],
                    ins=[cc_in[:].opt()],
                    outs=[cc_out[:].opt()],
                )
                sums_row = s0t.tile([1, 4], F32, name="sums_row")
                nc.sync.dma_start(sums_row, cc_out[:])
                sw_row = s0t.tile([1, 4], F32, name="sw_row")
                nc.vector.tensor_scalar(
                    out=sw_row, in0=sums_row, scalar1=1.0 / NW, scalar2=EPS,
                    op0=MULT, op1=ADD,
                )
                beta_row = s0t.tile([1, 4], F32, name="beta_row")
                nc.vector.reciprocal(beta_row, sw_row)
                row8 = s0t.tile([1, 8], F32, name="row8")
                nc.vector.tensor_scalar(
                    out=row8[:, 0:4], in0=beta_row, scalar1=ALPHA, scalar2=None,
                    op0=MULT, op1=mybir.AluOpType.bypass,
                )
                nc.vector.tensor_copy(row8[:, 4:8], sw_row)
                ps_b = psum.tile([P, 512], F32, name="ps_main")
                nc.tensor.matmul(
                    ps_b[:, :8], ones_row, row8, start=True, stop=True
                )
                nc.vector.tensor_copy(sb_scales, ps_b[:, :8])

            # ---------------- phase 0/1: x-quant + gate/up + h' ----------------
            with contextlib.ExitStack() as ph1:
                xqt_p = ph1.enter_context(tc.tile_pool(name="xqt", bufs=1))

                xqt = xqt_p.tile([P, KD, T], BF16)  # XqT: [dm-part, k, token]

                # x quantization (per token-tile) in its own pool scope
                with tc.tile_pool(name="xw", bufs=3) as xw_p:
                    for m in range(MT):
                        xt = xw_p.tile([P, DM], F32, name="xt")
                        nc.sync.dma_start(xt, x_d[m * P : (m + 1) * P, :])
                        amax = xw_p.tile([P, 1], F32, name="amax")
                        nc.vector.tensor_reduce(
                            amax, xt, axis=AXX, op=MAX, apply_absolute_value=True
                        )
                        nc.vector.tensor_scalar(
                            out=sx_all[:, m : m + 1], in0=amax, scalar1=EPS,
                            scalar2=1.0 / 127.0, op0=MAX, op1=MULT,
                        )
                        nc.vector.reciprocal(
                            rx_all[:, m : m + 1], sx_all[:, m : m + 1]
                        )
                        nc.vector.tensor_tensor(
                            out=sxg_all[:, m : m + 1], in0=sx_all[:, m : m + 1],
                            in1=sb_scales[:, 4:5], op=MULT,
                        )
                        nc.vector.tensor_tensor(
                            out=sxu_all[:, m : m + 1], in0=sx_all[:, m : m + 1],
                            in1=sb_scales[:, 5:6], op=MULT,
                        )
                        xr = xw_p.tile([P, DM], F32, name="xr")
                        nc.vector.tensor_scalar(
                            out=xr, in0=xt, scalar1=rx_all[:, m : m + 1], scalar2=CR,
                            op0=MULT, op1=ADD,
                        )
                        xq = xw_p.tile([P, DM], BF16, name="xq")
                        nc.vector.tensor_scalar(
                            out=xq, in0=xr, scalar1=CR, scalar2=None,
                            op0=SUB, op1=mybir.AluOpType.bypass,
                        )
                        nc.sync.dma_start_transpose(
                            xqt[:, :, m * P : (m + 1) * P], xq
                        )

                wraw_p = ph1.enter_context(tc.tile_pool(name="wraw", bufs=3))
                wtern_p = ph1.enter_context(tc.tile_pool(name="wtern", bufs=3))
                wchunk_p = ph1.enter_context(tc.tile_pool(name="wchunk", bufs=2))
                gate_p = ph1.enter_context(
                    tc.tile_pool(name="gate", bufs=2 * MT + 2)
                )
                hpr_p = ph1.enter_context(tc.tile_pool(name="hpr", bufs=4))
                sc_p = ph1.enter_context(tc.tile_pool(name="scp", bufs=2))

                # gate/up passes per 1024-ff group
                gate_tiles = {}
                for ng in range(NG):
                    for proj, wsrc, beta_col in ((0, wg_d, 0), (1, wu_d, 1)):
                        chunk = wchunk_p.tile([P, KD, 1024], BF16, name="wchunk")
                        for s8 in range(8):
                            s = ng * 8 + s8
                            raw = wraw_p.tile([P, DM], F32, name="wraw")
                            nc.sync.dma_start(raw, wsrc[s * P : (s + 1) * P, :])
                            nc.scalar.activation(
                                out=raw, in_=raw, func=AFT.Tanh,
                                scale=sb_scales[:, beta_col : beta_col + 1],
                            )
                            tern = wtern_p.tile([P, DM], BF16, name="wtern")
                            nc.vector.tensor_scalar(
                                out=tern, in0=raw, scalar1=CR, scalar2=CR,
                                op0=ADD, op1=SUB,
                            )
                            nc.sync.dma_start_transpose(
                                chunk[:, :, s8 * P : (s8 + 1) * P], tern
                            )
                        for mg0 in range(0, MT, MGP1):
                            mgn = min(MGP1, MT - mg0)
                            pss = [
                                [
                                    psum.tile([P, 512], F32, name="ps_main")
                                    for _ in range(2)
                                ]
                                for _ in range(mgn)
                            ]
                            for k in range(KD):
                                for m4 in range(mgn):
                                    m = mg0 + m4
                                    lhsT = xqt[:, k, m * P : (m + 1) * P]
                                    for j in range(2):
                                        nc.tensor.matmul(
                                            pss[m4][j],
                                            lhsT,
                                            chunk[:, k, j * 512 : (j + 1) * 512],
                                            start=(k == 0),
                                            stop=(k == KD - 1),
                                        )
                            for m4 in range(mgn):
                                m = mg0 + m4
                                for j in range(2):
                                    if proj == 0:
                                        gt = gate_p.tile([P, 512], F32, name="gate_t")
                                        nc.scalar.activation(
                                            out=gt, in_=pss[m4][j], func=AFT.Sigmoid,
                                            scale=sxg_all[:, m : m + 1],
                                        )
                                        gate_tiles[(m, j)] = gt
                                    else:
                                        hp = hpr_p.tile([P, 512], F32, name="hp")
                                        nc.vector.tensor_tensor(
                                            out=hp, in0=gate_tiles[(m, j)],
                                            in1=pss[m4][j], op=MULT,
                                        )
                                        nc.vector.tensor_reduce(
                                            accs[:, m, ng * 2 + j : ng * 2 + j + 1],
                                            hp, axis=AXX,
                                            op=MAX, apply_absolute_value=True,
                                        )
                                        nc.sync.dma_start(
                                            hp_d[
                                                m * P : (m + 1) * P,
                                                ng * 1024 + j * 512 : ng * 1024
                                                + (j + 1) * 512,
                                            ],
                                            hp,
                                        )

                # w_down ternarize into DRAM (natural layout)
                for sd in range(DM // P):
                    for c0 in range(0, FF, WPC3):
                        raw = wraw_p.tile([P, WPC3], F32, name="wraw")
                        nc.sync.dma_start(
                            raw, wd_d[sd * P : (sd + 1) * P, c0 : c0 + WPC3]
                        )
                        nc.scalar.activation(
                            out=raw, in_=raw, func=AFT.Tanh,
                            scale=sb_scales[:, 2:3],
                        )
                        ternd = wtern_p.tile([P, WPC3], BF16, name="wtern")
                        nc.vector.tensor_scalar(
                            out=ternd, in0=raw, scalar1=CR, scalar2=CR,
                            op0=ADD, op1=SUB,
                        )
                        nc.sync.dma_start(
                            wdt_d[sd * P : (sd + 1) * P, c0 : c0 + WPC3], ternd
                        )

                # h scales per token tile
                for m in range(MT):
                    am = sc_p.tile([P, 1], F32, name="am")
                    nc.vector.tensor_reduce(
                        am, accs[:, m, :], axis=AXX, op=MAX
                    )
                    nc.vector.tensor_tensor(
                        out=am, in0=am, in1=sxu_all[:, m : m + 1], op=MULT
                    )
                    sh = sc_p.tile([P, 1], F32, name="sh")
                    nc.vector.tensor_scalar(
                        out=sh, in0=am, scalar1=EPS, scalar2=1.0 / 127.0,
                        op0=MAX, op1=MULT,
                    )
                    rs = sc_p.tile([P, 1], F32, name="rs")
                    nc.vector.reciprocal(rs, sh)
                    nc.vector.tensor_tensor(
                        out=rph_all[:, m : m + 1], in0=rs,
                        in1=sxu_all[:, m : m + 1], op=MULT,
                    )
                    nc.vector.tensor_tensor(
                        out=shd_all[:, m : m + 1], in0=sh,
                        in1=sb_scales[:, 6:7], op=MULT,
                    )
                    nc.sync.dma_start(
                        shs_d[0, m * P : (m + 1) * P], shd_all[:, m : m + 1]
                    )

            # ---------------- S5: quantize h' ----------------
            with tc.tile_pool(name="s5", bufs=3) as s5p:
                for m in range(MT):
                    for c0 in range(0, FF, 2048):
                        hpt = s5p.tile([P, 2048], F32, name="hpt")
                        nc.sync.dma_start(
                            hpt, hp_d[m * P : (m + 1) * P, c0 : c0 + 2048]
                        )
                        nc.vector.tensor_scalar(
                            out=hpt, in0=hpt, scalar1=rph_all[:, m : m + 1],
                            scalar2=CR, op0=MULT, op1=ADD,
                        )
                        hqt = s5p.tile([P, 2048], BF16, name="hqt")
                        nc.vector.tensor_scalar(
                            out=hqt, in0=hpt, scalar1=CR, scalar2=None,
                            op0=SUB, op1=mybir.AluOpType.bypass,
                        )
                        nc.sync.dma_start(
                            hq_d[m * P : (m + 1) * P, c0 : c0 + 2048], hqt
                        )

            # ---------------- phase 3: down projection ----------------
            with contextlib.ExitStack() as ph3:
                wdt_p = ph3.enter_context(tc.tile_pool(name="wdtp", bufs=KG + 1))
                hqt_p = ph3.enter_context(tc.tile_pool(name="hqtp", bufs=KG + 1))
                acc_p = ph3.enter_context(tc.tile_pool(name="accp", bufs=MD))
                shs_p = ph3.enter_context(tc.tile_pool(name="shsp", bufs=1))
                fin_p = ph3.enter_context(tc.tile_pool(name="finp", bufs=2))

                shs_row = shs_p.tile([1, T], F32, name="shs_row")
                nc.sync.dma_start(shs_row, shs_d[:])
                shs_bc = shs_p.tile([P, T], F32, name="shs_bc")
                for t in range(NT3):
                    ps_bc = psum.tile([P, 512], F32, name="ps_main")
                    nc.tensor.matmul(
                        ps_bc[:, :TN], ones_row,
                        shs_row[:, t * TN : (t + 1) * TN], start=True, stop=True,
                    )
                    nc.vector.tensor_copy(
                        shs_bc[:, t * TN : (t + 1) * TN], ps_bc[:, :TN]
                    )

                out_acc = [acc_p.tile([P, T], F32, name="oacc") for _ in range(MD)]

                for kg in range(NKG):
                    wdt_tiles = []
                    hqt_tiles = []
                    for kk in range(KG):
                        k = kg * KG + kk
                        wt_t = wdt_p.tile([P, DM], BF16, name="wdt_t")
                        nc.sync.dma_start_transpose(
                            wt_t, wdt_d[:, k * P : (k + 1) * P]
                        )
                        wdt_tiles.append(wt_t)
                        hq_t = hqt_p.tile([P, T], BF16, name="hqt_t")
                        nc.sync.dma_start_transpose(
                            hq_t, hq_d[:, k * P : (k + 1) * P]
                        )
                        hqt_tiles.append(hq_t)
                    for mq0 in range(0, MD, MGP3):
                        mqn = min(MGP3, MD - mq0)
                        pss = [
                            [
                                psum.tile([P, 512], F32, name="ps_main")
                                for _ in range(NT3)
                            ]
                            for _ in range(mqn)
                        ]
                        for kk in range(KG):
                            for m4 in range(mqn):
                                m = mq0 + m4
                                lhsT = wdt_tiles[kk][:, m * P : (m + 1) * P]
                                for t in range(NT3):
                                    nc.tensor.matmul(
                                        pss[m4][t][:, :TN],
                                        lhsT,
                                        hqt_tiles[kk][:, t * TN : (t + 1) * TN],
                                        start=(kk == 0),
                                        stop=(kk == KG - 1),
                                    )
                        for m4 in range(mqn):
                            m = mq0 + m4
                            for t in range(NT3):
                                dst = out_acc[m][:, t * TN : (t + 1) * TN]
                                if kg == 0:
                                    nc.vector.tensor_copy(dst, pss[m4][t][:, :TN])
                                else:
                                    nc.vector.tensor_tensor(
                                        out=dst, in0=dst, in1=pss[m4][t][:, :TN],
                                        op=ADD,
                                    )

                for m in range(MD):
                    ot = fin_p.tile([P, T], F32, name="ot")
                    nc.vector.tensor_tensor(
                        out=ot, in0=out_acc[m], in1=shs_bc, op=MULT
                    )
                    nc.sync.dma_start(out_d[m * P : (m + 1) * P, :], ot)

    nc.compile()
    return nc


_CACHE = {}
TRACE = False  # set True (e.g. from test.py) to capture an NTFF profile
LAST_RESULTS = None


def _get_program(T, DM, FF, ncores, ff_sh, dm_sh):
    key = (T, DM, FF, ncores, ff_sh, dm_sh)
    if key not in _CACHE:
        _CACHE[key] = build_program(T, DM, FF, ncores, ff_sh, dm_sh)
    return _CACHE[key]


def kernel(x, w_gate, w_up, w_down):
    from concourse.bass_utils import run_bass_kernel_spmd

    x = np.asarray(x, dtype=np.float32)
    w_gate = np.ascontiguousarray(np.asarray(w_gate, dtype=np.float32))
    w_up = np.ascontiguousarray(np.asarray(w_up, dtype=np.float32))
    w_down = np.ascontiguousarray(np.asarray(w_down, dtype=np.float32))

    B, S, DM = x.shape
    FF = w_gate.shape[0]
    NCORES = 8
    NTOK = B * S
    T = NTOK // NCORES
    ff_sh = FF // NCORES
    dm_sh = DM // NCORES

    xf = np.ascontiguousarray(x.reshape(NTOK, DM))
    nc = _get_program(T, DM, FF, NCORES, ff_sh, dm_sh)

    in_maps = []
    for c in range(NCORES):
        in_maps.append(
            {
                "x": np.ascontiguousarray(xf[c * T : (c + 1) * T]),
                "wg": w_gate,
                "wu": w_up,
                "wd": w_down,
                "wg_sh": np.ascontiguousarray(w_gate[c * ff_sh : (c + 1) * ff_sh]),
                "wu_sh": np.ascontiguousarray(w_up[c * ff_sh : (c + 1) * ff_sh]),
                "wd_sh": np.ascontiguousarray(w_down[c * dm_sh : (c + 1) * dm_sh]),
            }
        )

    res = run_bass_kernel_spmd(
        nc, in_maps, core_ids=list(range(NCORES)), trace=TRACE
    )
    global LAST_RESULTS
    LAST_RESULTS = res
    out = np.empty((NTOK, DM), dtype=np.float32)
    for c in range(NCORES):
        out[c * T : (c + 1) * T] = res.results[c]["out_t"].T
    return out.reshape(B, S, DM)
